# revision 1
# baseline (speedup 1.0000x reference)
"""Multi-head self-attention (RoPE + softmax + out-proj) for Trainium2,
sharded over 8 NeuronCores: data-parallel over batch (4) x tensor-parallel
over heads (2 groups of 8). Each core computes q/k/v projections for its
head group, attention, and a partial output projection; the host sums the
two partials per batch and adds the bias.

Per-core layout highlights:
  - All matmul operands are float32r (rounded fp32), which streams at the
    full 1 cycle/row PE rate at N=512 (plain fp32 runs at 1/4 rate).
  - q/k are produced transposed ([head_dim, n]) by projecting against the
    Wq / Wkv column slices; RoPE's rotate_half is done with 32-partition
    shifted DVE copies, with the sign folded into a host-negated sin table.
  - Scores are computed transposed (S^T[m, n]) with K=64 row-group-packed
    matmul pairs (two heads concurrently in the PE array), so softmax's
    sum over keys m becomes a matmul reduction: v is extended with a ones
    column (M=65 matmul) whose output row 64 accumulates the denominator.
  - exp runs on the scalar engine straight out of PSUM in 1024-wide
    instructions; normalization uses a DVE reciprocal plus K=1 broadcast
    matmuls; the attention wave of pair p is software-pipelined with the
    projections of pair p+1 and the output projection of finished quarters
    so the tensor engine stays dense (HAM stays un-throttled).
"""

import numpy as np

import concourse.bass as bass
import concourse.mybir as mybir
import concourse.tile as tile

B, N, DIM, H, DH = 4, 2048, 1024, 16, 64
SCALE = DH**-0.5
N_CORES = 8
HG = 8  # heads per core
INNER = HG * DH  # 512, inner dim slice per core
PAIRS = INNER // 128  # 4 head pairs (=128-partition inner chunks)
NB = 4  # n blocks of 512
MB = 16  # m blocks of 128
KD = DIM // 128  # 8 contraction chunks

F32 = mybir.dt.float32
F32R = mybir.dt.float32r
EXP = mybir.ActivationFunctionType.Exp

MAX_WAITS = 1


def _split_excess_waits(nc):
    """This walrus build rejects >1 semaphore wait per instruction; hoist
    excess waits onto nops inserted before the instruction on its engine."""
    import bass_rust

    for f in nc.m.functions:
        for bb in f.blocks:
            il = bb.instructions
            i = 0
            while i < len(il):
                inst = il[i]
                si = inst.sync_info
                if si is not None and si.on_wait and len(si.on_wait) > MAX_WAITS:
                    waits = list(si.on_wait)
                    si.on_wait = waits[:MAX_WAITS]
                    rest = waits[MAX_WAITS:]
                    eng = nc.engines[inst.engine]
                    insert_at = i
                    for j in range(0, len(rest), MAX_WAITS):
                        b = eng.nop(nofuse=True, hint="wait_split")
                        ni = b.ins
                        tail = nc.cur_bb.bb.instructions
                        assert tail[-1] is ni
                        tail.pop()
                        nsi = ni.sync_info
                        if nsi is None:
                            ni.sync_info = bass_rust.SyncInfo(
                                on_wait=rest[j : j + MAX_WAITS], on_update=[]
                            )
                        else:
                            nsi.on_wait = rest[j : j + MAX_WAITS]
                        il.insert(insert_at, ni)
                        insert_at += 1
                        i += 1
                i += 1


class _FixedTileContext(tile.TileContext):
    def __exit__(self, exc_type, exc_val, exc_tb):
        res = super().__exit__(exc_type, exc_val, exc_tb)
        if exc_type is None:
            _split_excess_waits(self.nc)
        return res


def build_kernel():
    nc = bass.Bass()
    xT = nc.dram_tensor("xT", [DIM, N], F32, kind="ExternalInput")
    wq = nc.dram_tensor("wq", [DIM, INNER], F32, kind="ExternalInput")
    wk = nc.dram_tensor("wk", [DIM, INNER], F32, kind="ExternalInput")
    wv = nc.dram_tensor("wv", [DIM, INNER], F32, kind="ExternalInput")
    wo = nc.dram_tensor("wo", [INNER, DIM], F32, kind="ExternalInput")
    cosT = nc.dram_tensor("cosT", [128, N], F32, kind="ExternalInput")
    sinT = nc.dram_tensor("sinT", [128, N], F32, kind="ExternalInput")
    out = nc.dram_tensor("out", [N, DIM], F32, kind="ExternalOutput")

    vs = nc.dram_tensor("vs", [N, INNER], F32R)  # v bounce scratch
    xr = nc.dram_tensor("xr", [DIM, N], F32R)  # pre-cast x^T

    xTr = xr.rearrange("(c p) n -> p c n", p=128)

    with _FixedTileContext(nc) as tc:
        with (
            tc.tile_pool(name="const", bufs=1) as cpool,
            tc.tile_pool(name="qk", bufs=1) as qkpool,
            tc.tile_pool(name="ps", space=bass.MemorySpace.PSUM, bufs=1) as ps,
            tc.tile_pool(name="io", bufs=1) as iopool,
        ):
            # ---- constants ----
            cos_t = cpool.tile([128, N], F32, tag="cos")
            sin_t = cpool.tile([128, N], F32, tag="sin")
            nc.sync.dma_start(cos_t[:], cosT[:])
            nc.sync.dma_start(sin_t[:], sinT[:])
            ones_f = cpool.tile([128, 64], F32, tag="onesf")
            nc.vector.memset(ones_f[:], 1.0)
            onesr = cpool.tile([128, 64], F32R, tag="onesr")
            nc.vector.tensor_copy(onesr[:], ones_f[:])

            # ---- per-pair q/k projection blocks            # ---- per-pair q/k projection blocks (emitted interleaved with
            #      the previous pair's attention so the PE never idles) ----
            def proj_pair_blocks(p):
                csl = slice(p * 128, (p + 1) * 128)
                wt = {}

                def load_w():
                    for nm, wd in (("q", wq), ("k", wk)):
                        t = iopool.tile([128, KD, 128], F32R, tag=f"w{nm}", bufs=1, name=f"w{nm}_{p}")
                        nc.gpsimd.dma_start(
                            t[:], wd.rearrange("(c p) i -> p c i", p=128)[:, :, csl]
                        )
                        wt[nm] = t
                qT_t = qkpool.tile([128, N], F32R, tag="qT", bufs=2)
                kT_t = qkpool.tile([128, N], F32R, tag="kT", bufs=2)

                xts = {}

                def block(nb, pl, nm, tgt):
                    def emit():
                        if p == 0 and nb in first_x:
                            xts[nb] = first_x[nb]
                        if nb not in xts:
                            x_t = iopool.tile(
                                [128, KD, 512], F32R, tag="xv", bufs=2,
                                name=f"x_{p}_{nb}",
                            )
                            nc.sync.dma_start(
                                x_t[:], xTr[:, :, nb * 512 : (nb + 1) * 512]
                            )
                            xts[nb] = x_t
                        x_t = xts[nb]
                        nsl = slice(nb * 512, (nb + 1) * 512)
                        pq = ps.tile([128, 2, 512], F32, tag="s", bufs=3)
                        for dc in range(KD):
                            nc.tensor.matmul(
                                pq[:, 0, :], wt[nm][:, dc, :], x_t[:, dc, :],
                                start=(dc == 0), stop=(dc == KD - 1),
                            )
                        # rotate_half via 32-partition shifted copies; sign
                        # folded into sin_t (host negates low half rows)
                        tmp = iopool.tile([128, 512], F32, tag="tmp", bufs=2)
                        for g in range(4):
                            dst = slice(g * 32, (g + 1) * 32)
                            ssrc = slice((g ^ 1) * 32, ((g ^ 1) + 1) * 32)
                            nc.vector.tensor_copy(tmp[dst, :], pq[ssrc, 0, :])
                        nc.vector.tensor_mul(tmp[:], tmp[:], sin_t[:, nsl])
                        nc.vector.tensor_mul(tgt[:, nsl], pq[:, 0, :], cos_t[:, nsl])
                        nc.vector.tensor_add(tgt[:, nsl], tgt[:, nsl], tmp[:])
                    return emit

                blocks = []
                for nb in range(NB):
                    blocks.append(block(nb, 0, "q", qT_t))
                    blocks.append(block(nb, 1, "k", kT_t))
                return load_w, blocks, qT_t, kT_t

            load_w0, blocks0, qT0, kT0 = proj_pair_blocks(0)
            load_w0()

            def _pair0_emit(nb):
                blocks0[2 * nb]()
                blocks0[2 * nb + 1]()

            # ---- first pass over x: v projection (all heads) + pair-0 q/k ----
            pair0_hook = {"emit": _pair0_emit}
            with tc.tile_pool(name="vproj", bufs=1) as vpj:
              wv_t = vpj.tile([128, KD, INNER], F32R, tag="wv")
              wvr = wv.rearrange("(c p) i -> p c i", p=128)
              first_x = {}
              xTf = xT.rearrange("(c p) n -> p c n", p=128)
              for nb in range(NB):
                  xv_t = iopool.tile([128, KD, 512], F32R, tag="xv", bufs=2)
                  for dc in range(KD):
                      # interleave the wv chunks with the first x tile so the
                      # accumulation chain can start as soon as chunk 0 lands
                      if nb == 0:
                          nc.gpsimd.dma_start(wv_t[:, dc, :], wvr[:, dc, :])
                      nc.gpsimd.dma_start(
                          xv_t[:, dc, :],
                          xTf[:, dc, nb * 512 : (nb + 1) * 512],
                      )
                  first_x[nb] = xv_t
                  for sub in range(4):
                      pv = ps.tile([128, 512], F32, tag="s", bufs=3)
                      for dc in range(KD):
                          nc.tensor.matmul(
                              pv[:],
                              xv_t[:, dc, sub * 128 : (sub + 1) * 128],
                              wv_t[:, dc, :],
                              start=(dc == 0),
                              stop=(dc == KD - 1),
                          )
                      vstg = iopool.tile([128, 512], F32R, tag="vst", bufs=2)
                      nc.vector.tensor_copy(vstg[:], pv[:])
                      m0 = nb * 512 + sub * 128
                      nc.sync.dma_start(vs[m0 : m0 + 128, :], vstg[:])
                  pair0_hook["emit"](nb)
                  # write the already-cast x tile back to DRAM for the
                  # pair-1..3 projection passes (fast non-cast HWDGE)
                  nc.sync.dma_start(
                      xTr[:, :, nb * 512 : (nb + 1) * 512], xv_t[:]
                  )

            # pair-0 projections are emitted inside the first-pass loop via
            # pair0_hook (sharing its x tiles)
            pair_qk = {0: (qT0, kT0)}

            # ---- attention (pair p) interleaved with projections (p+1) ----
            with tc.tile_pool(name="attn", bufs=1) as at:
                otn = [
                    at.tile([128, 4, 512], F32R, tag=f"otn{p}", name=f"otn{p}")
                    for p in range(PAIRS)
                ]
                wo_h = []

                def load_wo():
                    for dh, wtag in ((0, "qT"), (1, "kT")):
                        woh = qkpool.tile(
                            [128, PAIRS, 512], F32R, tag=wtag, bufs=2,
                            name=f"wo_h{dh}",
                        )
                        nc.gpsimd.dma_start(
                            woh[:],
                            wo.rearrange("(c p) d -> p c d", p=128)[
                                :, :, dh * 512 : (dh + 1) * 512
                            ],
                        )
                        wo_h.append(woh)

                opq = []

                def outproj_block(nb, dh):
                    def emit():
                        q4, r4 = divmod(nb, 4)
                        nsl = slice(nb * 128, (nb + 1) * 128)
                        po = ps.tile([128, 2, 512], F32, tag="s", bufs=3)
                        for c in range(PAIRS):
                            nc.tensor.matmul(
                                po[:, 0, :],
                                otn[c][:, q4, r4 * 128 : (r4 + 1) * 128],
                                wo_h[dh][:, c, :],
                                start=(c == 0),
                                stop=(c == PAIRS - 1),
                            )
                        ost = iopool.tile([128, 512], F32, tag="ost", bufs=2)
                        nc.any.tensor_copy(ost[:], po[:, 0, :])
                        nc.sync.dma_start(
                            out[nsl, dh * 512 : (dh + 1) * 512], ost[:]
                        )
                    return emit

                def outproj_quarter(q4):
                    # queue this quarter's out-projection; drained one block
                    # at a time inside the next quarter's attention loop
                    for r4 in range(4):
                        for dh in range(2):
                            opq.append(outproj_block(q4 * 4 + r4, dh))

                def load_vext(p):
                    ves = []
                    for j in range(2):
                        h = 2 * p + j
                        ve = at.tile(
                            [128, MB, 65], F32R, tag="vext", bufs=4,
                            name=f"ve_{p}_{j}",
                        )
                        nc.sync.dma_start(
                            ve[:, :, 0:64],
                            vs.rearrange("(mb q) i -> q mb i", q=128)[
                                :, :, h * 64 : (h + 1) * 64
                            ],
                        )
                        for mb in range(MB):
                            nc.gpsimd.tensor_copy(ve[:, mb, 64:65], onesr[:, 0:1])
                        ves.append(ve)
                    return ves

                vext_next = load_vext(0)
                for p in range(PAIRS):
                    qT_t, kT_t = pair_qk.pop(p)
                    vext = vext_next
                    if p == PAIRS - 1:
                        load_wo()
                    if p + 1 < PAIRS:
                        load_wn, blocks_n, qTn, kTn = proj_pair_blocks(p + 1)
                        load_wn()
                        pair_qk[p + 1] = (qTn, kTn)
                        vext_next = load_vext(p + 1)
                    else:
                        blocks_n = []
                    blk_i = 0
                    for f in range(2):
                        for sub in range(2):
                            n0 = f * 1024 + sub * 512
                            ot_ab = [
                                ps.tile([128, 512], F32, tag="ot", bufs=2, name=f"ot{jj}")
                                for jj in range(2)
                            ]
                            for mb2 in range(MB // 2):
                                s_tiles = []
                                for j in range(2):
                                    psl = slice(64 * j, 64 * (j + 1))
                                    s_t = ps.tile([128, 2, 512], F32, tag="s", bufs=3, name=f"s{j}")
                                    for hm in range(2):
                                        mb = 2 * mb2 + hm
                                        msl = slice(mb * 128, (mb + 1) * 128)
                                        nc.tensor.matmul(
                                            s_t[:, hm, :],
                                            kT_t[psl, msl],
                                            qT_t[psl, n0 : n0 + 512],
                                            start=True,
                                            stop=True,
                                        )
                                    s_tiles.append(s_t)
                                pts = []
                                for j in range(2):
                                    pt = at.tile([128, 2, 512], F32R, tag="pt", bufs=5, name=f"pt{j}")
                                    nc.scalar.activation(
                                        pt[:], s_tiles[j][:], EXP, scale=SCALE
                                    )
                                    pts.append(pt)
                                for j in range(2):
                                    for hm in range(2):
                                        mb = 2 * mb2 + hm
                                        nc.tensor.matmul(
                                            ot_ab[j][0:65, :],
                                            vext[j][:, mb, :],
                                            pts[j][:, hm, :],
                                            start=(mb == 0),
                                            stop=(mb == MB - 1),
                                        )
                                # spread next pair's projection work through
                                # the attention chain to keep the PE dense
                                if mb2 % 2 == 1:
                                    if blk_i < len(blocks_n):
                                        blocks_n[blk_i]()
                                    blk_i += 1
                                    # in the last pair, spread the previous
                                    # quarter's output projection here too
                                    if opq:
                                        opq.pop(0)()
                                    if len(opq) > 4:
                                        opq.pop(0)()
                            # spill OT accumulators to SBUF (frees the
                            # psum banks for the next quarter immediately)
                            osb = at.tile([65, 2, 512], F32, tag="ots", bufs=4)
                            nc.vector.tensor_copy(osb[:, 0, :], ot_ab[0][0:65, :])
                            nc.vector.tensor_copy(osb[:, 1, :], ot_ab[1][0:65, :])
                            # denominators -> recip -> bcast -> normalize
                            rin = at.tile([33, 512], F32, tag="rin", bufs=2)
                            nc.vector.tensor_copy(rin[0:1, :], osb[64:65, 0, :])
                            nc.vector.tensor_copy(rin[32:33, :], osb[64:65, 1, :])
                            rec = at.tile([33, 512], F32R, tag="rec", bufs=2)
                            with nc.allow_low_precision(
                                reason="f32r reciprocal for softmax denom"
                            ):
                                # one op covers rows 0..32; rows 1-31 junk
                                nc.vector.reciprocal(rec[:], rin[:])
                            for j in range(2):
                                row = 32 * j
                                bc = ps.tile(
                                    [128, 512], F32, tag="ot", bufs=2,
                                    name=f"bc{j}",
                                )
                                nc.tensor.matmul(
                                    bc[0:64, :],
                                    onesr[row : row + 1, :],
                                    rec[row : row + 1, :],
                                    start=True,
                                    stop=True,
                                )
                                nc.vector.tensor_mul(
                                    otn[p][64 * j : 64 * (j + 1), f * 2 + sub, :],
                                    osb[0:64, j, :],
                                    bc[0:64, :],
                                )
                            if p == PAIRS - 1:
                                outproj_quarter(f * 2 + sub)
                                if f == 1 and sub == 1:
                                    while opq:
                                        opq.pop(0)()

    return nc


_CACHED = {}


def _get_kernel():
    if "nc" not in _CACHED:
        _CACHED["nc"] = build_kernel()
    return _CACHED["nc"]


def kernel(x, rotary_emb_x, Wq, Wkv, Wo, bo):
    from concourse.bass_utils import run_bass_kernel_spmd

    x = np.asarray(x, np.float32)
    rope = np.asarray(rotary_emb_x, np.float32)
    Wq = np.asarray(Wq, np.float32)
    Wkv = np.asarray(Wkv, np.float32)
    Wo = np.asarray(Wo, np.float32)
    bo = np.asarray(bo, np.float32)

    cosT = np.ascontiguousarray(np.cos(rope).T)  # [64, N]
    sinT = np.ascontiguousarray(np.sin(rope).T)
    cosT2 = np.ascontiguousarray(np.concatenate([cosT, cosT], axis=0))
    sinT2 = np.concatenate([sinT, sinT], axis=0)
    # fold rotate_half's sign into sin: the low half of each 64-row head
    # block multiplies -q_hi
    sinT2 = sinT2.copy()
    sinT2[0:32] = -sinT2[0:32]
    sinT2[64:96] = -sinT2[64:96]
    sinT2 = np.ascontiguousarray(sinT2)

    Wk_full = Wkv[:, : H * DH]
    Wv_full = Wkv[:, H * DH :]

    xTs = [np.ascontiguousarray(x[b].T) for b in range(B)]
    in_maps = []
    for core in range(N_CORES):
        b, hg = divmod(core, 2)
        isl = slice(hg * INNER, (hg + 1) * INNER)
        in_maps.append(
            {
                "xT": xTs[b],
                "wq": np.ascontiguousarray(Wq[:, isl]),
                "wk": np.ascontiguousarray(Wk_full[:, isl]),
                "wv": np.ascontiguousarray(Wv_full[:, isl]),
                "wo": np.ascontiguousarray(Wo[isl, :]),
                "cosT": cosT2,
                "sinT": sinT2,
            }
        )

    nc = _get_kernel()
    _CACHED["in_maps"] = in_maps
    res = run_bass_kernel_spmd(nc, in_maps, list(range(N_CORES)))
    outs = [res.results[i]["out"] for i in range(N_CORES)]
    full = np.stack(
        [outs[2 * b] + outs[2 * b + 1] + bo for b in range(B)], axis=0
    )
    return full



# revision 9
# speedup vs baseline: 1.2109x; 1.2109x over previous
"""Multi-head self-attention (RoPE + softmax + out-proj) for Trainium2,
sharded over 8 NeuronCores: data-parallel over batch (4) x tensor-parallel
over heads (2 groups of 8). Each core computes q/k/v projections for its
head group, attention, and a partial output projection; the host sums the
two partials per batch and adds the bias.

v2 design (bf16 datapath, resident operands, balanced engines):
  - All matmul operands are bf16 (same PE stream rate as f32r, half the
    SBUF/DMA bytes; DVE elementwise gets the 2x/4x 16-bit perf modes).
  - x^T stays resident in SBUF (loaded once; no DRAM re-reads per pair);
    v is projected once into a per-head SBUF layout [key, head, 65] whose
    65th column is a ones column, so the attention matmul accumulates the
    softmax denominator for free (M=65 matmuls).
  - RoPE's rotate_half is four 32-partition shifted bf16 copies (4x DVE
    mode) off a single PSUM evacuation; the sign is folded into a
    host-negated sin table.
  - Scores are computed transposed (S^T[keys, queries]) with K=64 matmuls
    row-group-packed two heads at a time; exp runs on the scalar engine
    straight out of PSUM in 1024-wide instructions (scale folded in).
  - Softmax normalization: reciprocal_approx_fast on the denominator rows,
    one K=2 broadcast matmul per quarter to spread both heads' reciprocals
    across 128 partitions, and the PSUM evacuation of the attention output
    is fused with the normalize multiply (no staging spill).
  - Software pipeline: per 512-query quarter, 8 slots of
    {scores, exp, av(lagged one quarter), filler}; fillers carry the next
    pair's projections, the v projection (first quarter), and the output
    projection (last pair), keeping the PE dense so HAM stays warm.
"""

import numpy as np

import concourse.bass as bass
import concourse.mybir as mybir
import concourse.tile as tile

B, N, DIM, H, DH = 4, 2048, 1024, 16, 64
SCALE = DH**-0.5
N_CORES = 8
HG = 8  # heads per core
INNER = HG * DH  # 512
PAIRS = 4  # head pairs per core
NB = 4  # 512-wide query/key blocks
MB = 16  # 128-wide key blocks
KD = DIM // 128  # contraction chunks

F32 = mybir.dt.float32
F32R = mybir.dt.float32r
BF16 = mybir.dt.bfloat16
I32 = mybir.dt.int32
EXP = mybir.ActivationFunctionType.Exp

# Schraudolph exp offload to the vector engine: set of (p, qi, mb2, j)
# score tiles whose exp is computed as bitcast(int32(x*A + B)) on the DVE
# instead of the scalar engine (which is otherwise the bottleneck).
SCHRAUDOLPH = set()
SCH_A = SCALE * (1 << 23) / np.log(2.0)
SCH_B = float(127 * (1 << 23)) - 366393.0

MAX_WAITS = 1


def _split_excess_waits(nc):
    """This walrus build rejects >1 semaphore wait per instruction; hoist
    excess waits onto nops inserted before the instruction on its engine."""
    import bass_rust

    for f in nc.m.functions:
        for bb in f.blocks:
            il = bb.instructions
            i = 0
            while i < len(il):
                inst = il[i]
                si = inst.sync_info
                if si is not None and si.on_wait and len(si.on_wait) > MAX_WAITS:
                    waits = list(si.on_wait)
                    si.on_wait = waits[:MAX_WAITS]
                    rest = waits[MAX_WAITS:]
                    eng = nc.engines[inst.engine]
                    insert_at = i
                    for j in range(0, len(rest), MAX_WAITS):
                        b = eng.nop(nofuse=True, hint="wait_split")
                        ni = b.ins
                        tail = nc.cur_bb.bb.instructions
                        assert tail[-1] is ni
                        tail.pop()
                        nsi = ni.sync_info
                        if nsi is None:
                            ni.sync_info = bass_rust.SyncInfo(
                                on_wait=rest[j : j + MAX_WAITS], on_update=[]
                            )
                        else:
                            nsi.on_wait = rest[j : j + MAX_WAITS]
                        il.insert(insert_at, ni)
                        insert_at += 1
                        i += 1
                i += 1


class _FixedTileContext(tile.TileContext):
    def __exit__(self, exc_type, exc_val, exc_tb):
        res = super().__exit__(exc_type, exc_val, exc_tb)
        if exc_type is None:
            _split_excess_waits(self.nc)
        return res


def build_kernel():
    nc = bass.Bass()
    xT = nc.dram_tensor("xT", [DIM, N], BF16, kind="ExternalInput")
    wq = nc.dram_tensor("wq", [DIM, INNER], BF16, kind="ExternalInput")
    wk = nc.dram_tensor("wk", [DIM, INNER], BF16, kind="ExternalInput")
    wv = nc.dram_tensor("wv", [DIM, INNER], BF16, kind="ExternalInput")
    wo = nc.dram_tensor("wo", [INNER, DIM], BF16, kind="ExternalInput")
    cosT = nc.dram_tensor("cosT", [128, N], BF16, kind="ExternalInput")
    sinT = nc.dram_tensor("sinT", [128, N], BF16, kind="ExternalInput")
    onesd = nc.dram_tensor("onesd", [64, 128], F32R, kind="ExternalInput")
    out = nc.dram_tensor("out", [N, DIM], F32, kind="ExternalOutput")

    xTr = xT.rearrange("(c p) n -> p c n", p=128)
    wor = wo.rearrange("(c p) d -> p c d", p=128)

    with _FixedTileContext(nc) as tc:
        with (
            tc.tile_pool(name="const", bufs=1) as cpool,
            tc.tile_pool(name="w", bufs=2) as wpool,
            tc.tile_pool(name="qk", bufs=2) as qkpool,
            tc.tile_pool(name="rope", bufs=3) as rpool,
            tc.tile_pool(name="pt", bufs=20) as ptpool,
            tc.tile_pool(name="at", bufs=1) as at,
            tc.tile_pool(name="io", bufs=1) as iopool,
            tc.tile_pool(name="ps", space=bass.MemorySpace.PSUM, bufs=1) as ps,
        ):
            # ---- resident constants / activations ----
            cos_t = cpool.tile([128, N], BF16, tag="cos")
            sin_t = cpool.tile([128, N], BF16, tag="sin")
            nc.sync.dma_start(cos_t[:], cosT[:])
            nc.sync.dma_start(sin_t[:], sinT[:])

            x_sb = cpool.tile([128, KD, N], BF16, tag="x")
            for dc in range(KD):
                nc.sync.dma_start(x_sb[:, dc, :], xTr[:, dc, :])

            load_w0_done = []

            # v resident per (key-block, head, dh+ones): [128, 16, 8, 65].
            # Memset the whole tile to 1.0 up front: the projection evacs
            # overwrite columns 0-63 of each head, leaving column 64 as the
            # ones column that accumulates the softmax denominator.
            v_sb = cpool.tile([128, MB, HG, DH + 1], BF16, tag="vsb")
            nc.vector.memset(v_sb[:], 1.0)

            # broadcast-matmul stationary, host-built (row0 spreads to
            # output partitions 0-63, row32 to 64-127; other rows are zero)
            onesblk = cpool.tile([64, 128], F32R, tag="onesblk")
            nc.sync.dma_start(onesblk[:], onesd[:])
            # persistent normalize staging: denominator rows 0 and 32 (other
            # rows stay 1.0 so the zero-weight broadcast rows see finite
            # values -- never NaN*0) and the Newton-iteration scratch
            den = at.tile([64, 512], F32, tag="den")
            nc.vector.memset(den[:], 1.0)
            rcp0 = at.tile([64, 512], I32, tag="rcp0")
            rcp_t = at.tile([64, 512], F32, tag="rcpt")
            rcp_u = at.tile([64, 512], F32, tag="rcpu")
            rcp1 = at.tile([64, 512], F32, tag="rcp1")
            rcp2 = at.tile([64, 512], F32R, tag="rcp2")

            # ---- per-pair q/k weight loads ----
            wtiles = {}

            def load_w(p):
                csl = slice(p * 128, (p + 1) * 128)
                ts = {}
                for nm, wd in (("q", wq), ("k", wk)):
                    t = wpool.tile([128, KD, 128], BF16, tag=f"w{nm}")
                    nc.gpsimd.dma_start(
                        t[:], wd.rearrange("(c p) i -> p c i", p=128)[:, :, csl]
                    )
                    ts[nm] = t
                wtiles[p] = ts

            # ---- projection block: qT/kT[:, nb*512:(nb+1)*512] for pair p ----
            def proj_block(p, nm, nb, tgt):
                def emit():
                    nsl = slice(nb * 512, (nb + 1) * 512)
                    pq = ps.tile([128, 512], F32, tag="pq", bufs=2)
                    wt = wtiles[p][nm]
                    for dc in range(KD):
                        nc.tensor.matmul(
                            pq[:], wt[:, dc, :], x_sb[:, dc, nsl],
                            start=(dc == 0), stop=(dc == KD - 1),
                        )
                    qsb = rpool.tile([128, 512], BF16, tag="qsb")
                    nc.vector.tensor_copy(qsb[:], pq[:])
                    # rotate_half: swap 32-row halves within each 64-row head
                    # block (sign folded into the host-negated sin table)
                    tmp = rpool.tile([128, 512], BF16, tag="tmp")
                    for g in range(4):
                        dst = slice(g * 32, (g + 1) * 32)
                        src = slice((g ^ 1) * 32, ((g ^ 1) + 1) * 32)
                        nc.vector.tensor_copy(tmp[dst, :], qsb[src, :])
                    nc.vector.tensor_mul(tmp[:], tmp[:], sin_t[:, nsl])
                    nc.vector.tensor_mul(tgt[:, nsl], qsb[:], cos_t[:, nsl])
                    nc.vector.tensor_add(tgt[:, nsl], tgt[:, nsl], tmp[:])
                return emit

            # ---- v projection block: keys [i*128, (i+1)*128) for all heads ----
            def v_block(i):
                def emit():
                    msl = slice(i * 128, (i + 1) * 128)
                    pv = ps.tile([128, 512], F32, tag="pq", bufs=2)
                    for dc in range(KD):
                        nc.tensor.matmul(
                            pv[:], x_sb[:, dc, msl], wv_t[:, dc, :],
                            start=(dc == 0), stop=(dc == KD - 1),
                        )
                    nc.vector.tensor_copy(v_sb[:, i, :, 0:DH], pv[:])
                return emit

            # ---- output projection block (one 128-query row block, one
            #      512-wide dim half) ----
            otn = [
                at.tile([128, NB, 512], BF16, tag=f"otn{p}", name=f"otn{p}")
                for p in range(PAIRS)
            ]

            def outproj_block(nb, dh):
                def emit():
                    q4, r4 = divmod(nb, 4)
                    nsl = slice(nb * 128, (nb + 1) * 128)
                    po = ps.tile([128, 512], F32, tag="pq", bufs=2)
                    for c in range(PAIRS):
                        nc.tensor.matmul(
                            po[:],
                            otn[c][:, q4, r4 * 128 : (r4 + 1) * 128],
                            wo_t[:, c, dh * 512 : (dh + 1) * 512],
                            start=(c == 0), stop=(c == PAIRS - 1),
                        )
                    ost = iopool.tile([128, 512], F32, tag="ost", bufs=3)
                    nc.vector.tensor_copy(ost[:], po[:])
                    nc.sync.dma_start(out[nsl, dh * 512 : (dh + 1) * 512], ost[:])
                return emit

            # ---- attention pipeline ----
            fillers = []
            avq = []

            def drain_fillers(k):
                for _ in range(k):
                    if fillers:
                        fillers.pop(0)()

            def drain_avq(cap, lag=0):
                pops = max(0, min(cap, len(avq) - lag))
                for _ in range(pops):
                    avq.pop(0)()

            def normalize(p, qi, qd):
                # stage both heads' denominator rows at partitions 0 and 32,
                # take the reciprocal there with a bit-magic seed plus two
                # Newton steps (stock DVE ops only; the per-op cost is
                # per-lane so the 33-row tile costs the same as one row),
                # then spread both rows across 128 partitions with one K=64
                # broadcast matmul.
                ot0, ot1 = qd["ot"]
                nc.vector.tensor_copy(den[0:1, :], ot0[64:65, :])
                nc.vector.tensor_copy(den[32:33, :], ot1[64:65, :])
                sl = slice(0, 33)
                nc.vector.tensor_scalar(
                    rcp0[sl, :], den[sl, :].bitcast(I32), 0x7EF312AC, -1,
                    mybir.AluOpType.subtract, mybir.AluOpType.mult,
                )
                r0 = rcp0[sl, :].bitcast(F32)
                nc.vector.tensor_mul(rcp_t[sl, :], den[sl, :], r0)
                nc.vector.tensor_scalar(
                    rcp_u[sl, :], rcp_t[sl, :], -1.0, 2.0,
                    mybir.AluOpType.mult, mybir.AluOpType.add,
                )
                nc.vector.tensor_mul(rcp1[sl, :], r0, rcp_u[sl, :])
                nc.vector.tensor_mul(rcp_t[sl, :], den[sl, :], rcp1[sl, :])
                nc.vector.tensor_scalar(
                    rcp_u[sl, :], rcp_t[sl, :], -1.0, 2.0,
                    mybir.AluOpType.mult, mybir.AluOpType.add,
                )
                nc.vector.tensor_mul(rcp2[sl, :], rcp1[sl, :], rcp_u[sl, :])
                bc = ps.tile([128, 512], F32, tag="pq", bufs=2)
                nc.tensor.matmul(
                    bc[:], onesblk[0:33, :], rcp2[0:33, :],
                    start=True, stop=True,
                )
                bcsb = at.tile([128, 512], F32, tag="bcsb", bufs=2)
                nc.vector.tensor_copy(bcsb[:], bc[:])
                # fused PSUM-evacuate + normalize (otn rows 0-63 = head 2p,
                # rows 64-127 = head 2p+1)
                nc.vector.tensor_mul(
                    otn[p][0:64, qi, :], ot0[0:64, :], bcsb[0:64, :]
                )
                nc.vector.tensor_mul(
                    otn[p][64:128, qi, :], ot1[0:64, :], bcsb[64:128, :]
                )

            def av_group(p, qi, mb2, qd):
                def emit():
                    if mb2 == 0:
                        qd["ot"] = [
                            ps.tile([128, 512], F32, tag="ot", bufs=2, name=f"ot{j}")
                            for j in range(2)
                        ]
                    pts = qd.pop(("pt", mb2))
                    for j in range(2):
                        pt, is_sch = pts[j]
                        for hm in range(2):
                            mb = 2 * mb2 + hm
                            rhs = pt[:, hm, :]
                            if is_sch:
                                rhs = rhs.bitcast(F32R)
                            nc.tensor.matmul(
                                qd["ot"][j][0:65, :],
                                v_sb[:, mb, 2 * p + j, :],
                                rhs,
                                start=(mb == 0), stop=(mb == MB - 1),
                            )
                    if mb2 == 7:
                        normalize(p, qi, qd)
                        if p == PAIRS - 1:
                            for r4 in range(4):
                                for dh in range(2):
                                    fillers.append(outproj_block(qi * 4 + r4, dh))
                return emit

            # prologue: pair-0 weights first on the gpsimd queue, then the
            # bulkier v/out weights
            load_w(0)
            wv_t = cpool.tile([128, KD, INNER], BF16, tag="wv")
            nc.gpsimd.dma_start(wv_t[:], wv.rearrange("(c p) i -> p c i", p=128))
            wo_t = cpool.tile([128, PAIRS, DIM], BF16, tag="wo")
            nc.gpsimd.dma_start(wo_t[:], wor[:])
            qT = {0: qkpool.tile([128, N], BF16, tag="qT", name="qT0")}
            kT = {0: qkpool.tile([128, N], BF16, tag="kT", name="kT0")}
            for nb in range(NB):
                proj_block(0, "k", nb, kT[0])()
            proj_block(0, "q", 0, qT[0])()
            fillers.extend(v_block(i) for i in range(MB))
            fillers.extend(proj_block(0, "q", nb, qT[0]) for nb in range(1, NB))

            for p in range(PAIRS):
                if p + 1 < PAIRS:
                    load_w(p + 1)
                    qT[p + 1] = qkpool.tile([128, N], BF16, tag="qT", name=f"qT{p+1}")
                    kT[p + 1] = qkpool.tile([128, N], BF16, tag="kT", name=f"kT{p+1}")
                    fillers.extend(
                        proj_block(p + 1, "k", nb, kT[p + 1]) for nb in range(NB)
                    )
                    fillers.extend(
                        proj_block(p + 1, "q", nb, qT[p + 1]) for nb in range(NB)
                    )
                for qi in range(4):
                    n0 = qi * 512
                    qd = {}
                    for mb2 in range(MB // 2):
                        pts = []
                        for j in range(2):
                            psl = slice(64 * j, 64 * (j + 1))
                            s_t = ps.tile([128, 2, 512], F32, tag="s", bufs=2, name=f"s{j}")
                            for hm in range(2):
                                mb = 2 * mb2 + hm
                                msl = slice(mb * 128, (mb + 1) * 128)
                                nc.tensor.matmul(
                                    s_t[:, hm, :],
                                    kT[p][psl, msl],
                                    qT[p][psl, n0 : n0 + 512],
                                    start=True, stop=True,
                                )
                            is_sch = (p, qi, mb2, j) in SCHRAUDOLPH
                            if is_sch:
                                pt = ptpool.tile(
                                    [128, 2, 512], I32, tag="pti", bufs=6
                                )
                                nc.vector.tensor_scalar(
                                    pt[:], s_t[:], SCH_A, SCH_B,
                                    mybir.AluOpType.mult, mybir.AluOpType.add,
                                )
                            else:
                                pt = ptpool.tile([128, 2, 512], BF16, tag="pt")
                                nc.scalar.activation(pt[:], s_t[:], EXP, scale=SCALE)
                            pts.append((pt, is_sch))
                        qd[("pt", mb2)] = pts
                        avq.append(av_group(p, qi, mb2, qd))
                        # interleave: prologue drains fast, steady state slow
                        if p == 0 and qi == 0:
                            drain_fillers(3)
                        elif p == PAIRS - 1:
                            drain_fillers(1)
                        elif mb2 % 3 == 2:
                            drain_fillers(1)
                        if p == PAIRS - 1:
                            drain_avq(2, lag=1)
                        else:
                            drain_avq(1, lag=3)

            # tail: remaining attention groups, then output projection
            drain_avq(len(avq))
            drain_fillers(len(fillers))
            assert not avq and not fillers

    return nc


_CACHED = {}


def _get_kernel():
    if "nc" not in _CACHED:
        _CACHED["nc"] = build_kernel()
    return _CACHED["nc"]


def kernel(x, rotary_emb_x, Wq, Wkv, Wo, bo):
    import ml_dtypes
    from concourse.bass_utils import run_bass_kernel_spmd

    BF = ml_dtypes.bfloat16
    x = np.asarray(x, np.float32)
    rope = np.asarray(rotary_emb_x, np.float32)
    Wq = np.asarray(Wq, np.float32)
    Wkv = np.asarray(Wkv, np.float32)
    Wo = np.asarray(Wo, np.float32)
    bo = np.asarray(bo, np.float32)

    cosT = np.cos(rope).T  # [64, N]
    sinT = np.sin(rope).T
    cosT2 = np.concatenate([cosT, cosT], axis=0)
    sinT2 = np.concatenate([sinT, sinT], axis=0).copy()
    # fold rotate_half's sign into sin: the low half of each 64-row head
    # block multiplies -q_hi
    sinT2[0:32] = -sinT2[0:32]
    sinT2[64:96] = -sinT2[64:96]
    cosT2 = np.ascontiguousarray(cosT2.astype(BF))
    sinT2 = np.ascontiguousarray(sinT2.astype(BF))

    Wk_full = Wkv[:, : H * DH]
    Wv_full = Wkv[:, H * DH :]

    onesd = np.zeros((64, 128), np.float32)
    onesd[0, 0:64] = 1.0
    onesd[32, 64:128] = 1.0

    xTs = [np.ascontiguousarray(x[b].T.astype(BF)) for b in range(B)]
    in_maps = []
    for core in range(N_CORES):
        b, hg = divmod(core, 2)
        isl = slice(hg * INNER, (hg + 1) * INNER)
        in_maps.append(
            {
                "xT": xTs[b],
                "wq": np.ascontiguousarray(Wq[:, isl].astype(BF)),
                "wk": np.ascontiguousarray(Wk_full[:, isl].astype(BF)),
                "wv": np.ascontiguousarray(Wv_full[:, isl].astype(BF)),
                "wo": np.ascontiguousarray(Wo[isl, :].astype(BF)),
                "cosT": cosT2,
                "sinT": sinT2,
                "onesd": onesd,
            }
        )

    nc = _get_kernel()
    _CACHED["in_maps"] = in_maps
    res = run_bass_kernel_spmd(nc, in_maps, list(range(N_CORES)))
    outs = [res.results[i]["out"] for i in range(N_CORES)]
    full = np.stack(
        [outs[2 * b] + outs[2 * b + 1] + bo for b in range(B)], axis=0
    )
    return full


# revision 10
# speedup vs baseline: 1.2328x; 1.0182x over previous
"""Multi-head self-attention (RoPE + softmax + out-proj) for Trainium2,
sharded over 8 NeuronCores: data-parallel over batch (4) x tensor-parallel
over heads (2 groups of 8). Each core computes q/k/v projections for its
head group, attention, and a partial output projection; the host sums the
two partials per batch and adds the bias.

v2 design (bf16 datapath, resident operands, balanced engines):
  - All matmul operands are bf16 (same PE stream rate as f32r, half the
    SBUF/DMA bytes; DVE elementwise gets the 2x/4x 16-bit perf modes).
  - x^T stays resident in SBUF (loaded once; no DRAM re-reads per pair);
    v is projected once into a per-head SBUF layout [key, head, 65] whose
    65th column is a ones column, so the attention matmul accumulates the
    softmax denominator for free (M=65 matmuls).
  - RoPE's rotate_half is four 32-partition shifted bf16 copies (4x DVE
    mode) off a single PSUM evacuation; the sign is folded into a
    host-negated sin table.
  - Scores are computed transposed (S^T[keys, queries]) with K=64 matmuls
    row-group-packed two heads at a time; exp runs on the scalar engine
    straight out of PSUM in 1024-wide instructions (scale folded in).
  - Softmax normalization: reciprocal_approx_fast on the denominator rows,
    one K=2 broadcast matmul per quarter to spread both heads' reciprocals
    across 128 partitions, and the PSUM evacuation of the attention output
    is fused with the normalize multiply (no staging spill).
  - Software pipeline: per 512-query quarter, 8 slots of
    {scores, exp, av(lagged one quarter), filler}; fillers carry the next
    pair's projections, the v projection (first quarter), and the output
    projection (last pair), keeping the PE dense so HAM stays warm.
"""

import numpy as np

import concourse.bass as bass
import concourse.mybir as mybir
import concourse.tile as tile

B, N, DIM, H, DH = 4, 2048, 1024, 16, 64
SCALE = DH**-0.5
N_CORES = 8
HG = 8  # heads per core
INNER = HG * DH  # 512
PAIRS = 4  # head pairs per core
NB = 4  # 512-wide query/key blocks
MB = 16  # 128-wide key blocks
KD = DIM // 128  # contraction chunks

F32 = mybir.dt.float32
F32R = mybir.dt.float32r
BF16 = mybir.dt.bfloat16
I32 = mybir.dt.int32
EXP = mybir.ActivationFunctionType.Exp

# Schraudolph exp offload to the vector engine: set of (p, qi, mb2, j)
# score tiles whose exp is computed as bitcast(int32(x*A + B)) on the DVE
# instead of the scalar engine (which is otherwise the bottleneck).
SCHRAUDOLPH = set()
SCH_A = SCALE * (1 << 23) / np.log(2.0)
SCH_B = float(127 * (1 << 23)) - 366393.0

MAX_WAITS = 1


def _split_excess_waits(nc):
    """This walrus build rejects >1 semaphore wait per instruction; hoist
    excess waits onto nops inserted before the instruction on its engine."""
    import bass_rust

    for f in nc.m.functions:
        for bb in f.blocks:
            il = bb.instructions
            i = 0
            while i < len(il):
                inst = il[i]
                si = inst.sync_info
                if si is not None and si.on_wait and len(si.on_wait) > MAX_WAITS:
                    waits = list(si.on_wait)
                    si.on_wait = waits[:MAX_WAITS]
                    rest = waits[MAX_WAITS:]
                    eng = nc.engines[inst.engine]
                    insert_at = i
                    for j in range(0, len(rest), MAX_WAITS):
                        b = eng.nop(nofuse=True, hint="wait_split")
                        ni = b.ins
                        tail = nc.cur_bb.bb.instructions
                        assert tail[-1] is ni
                        tail.pop()
                        nsi = ni.sync_info
                        if nsi is None:
                            ni.sync_info = bass_rust.SyncInfo(
                                on_wait=rest[j : j + MAX_WAITS], on_update=[]
                            )
                        else:
                            nsi.on_wait = rest[j : j + MAX_WAITS]
                        il.insert(insert_at, ni)
                        insert_at += 1
                        i += 1
                i += 1


class _FixedTileContext(tile.TileContext):
    def __exit__(self, exc_type, exc_val, exc_tb):
        res = super().__exit__(exc_type, exc_val, exc_tb)
        if exc_type is None:
            _split_excess_waits(self.nc)
        return res


def build_kernel():
    nc = bass.Bass()
    xT = nc.dram_tensor("xT", [DIM, N], BF16, kind="ExternalInput")
    wq = nc.dram_tensor("wq", [DIM, INNER], BF16, kind="ExternalInput")
    wk = nc.dram_tensor("wk", [DIM, INNER], BF16, kind="ExternalInput")
    wv = nc.dram_tensor("wv", [DIM, INNER], BF16, kind="ExternalInput")
    wo = nc.dram_tensor("wo", [INNER, DIM], BF16, kind="ExternalInput")
    cosT = nc.dram_tensor("cosT", [128, N], BF16, kind="ExternalInput")
    sinT = nc.dram_tensor("sinT", [128, N], BF16, kind="ExternalInput")
    onesd = nc.dram_tensor("onesd", [64, 128], F32R, kind="ExternalInput")
    out = nc.dram_tensor("out", [N, DIM], F32, kind="ExternalOutput")

    xTr = xT.rearrange("(c p) n -> p c n", p=128)
    wor = wo.rearrange("(c p) d -> p c d", p=128)

    with _FixedTileContext(nc) as tc:
        with (
            tc.tile_pool(name="const", bufs=1) as cpool,
            tc.tile_pool(name="w", bufs=2) as wpool,
            tc.tile_pool(name="qk", bufs=2) as qkpool,
            tc.tile_pool(name="rope", bufs=3) as rpool,
            tc.tile_pool(name="pt", bufs=20) as ptpool,
            tc.tile_pool(name="at", bufs=1) as at,
            tc.tile_pool(name="io", bufs=1) as iopool,
            tc.tile_pool(name="ps", space=bass.MemorySpace.PSUM, bufs=1) as ps,
        ):
            # ---- resident constants / activations ----
            # x first, split across the two HWDGE queues (sync + scalar) so
            # the projection chains aren't gated on one serialized queue
            x_sb = cpool.tile([128, KD, N], BF16, tag="x")
            for dc in range(KD):
                eng = nc.sync if dc % 2 == 0 else nc.scalar
                eng.dma_start(x_sb[:, dc, 0:1024], xTr[:, dc, 0:1024])
                eng.dma_start(x_sb[:, dc, 1024:N], xTr[:, dc, 1024:N])
            cos_t = cpool.tile([128, N], BF16, tag="cos")
            sin_t = cpool.tile([128, N], BF16, tag="sin")
            nc.sync.dma_start(cos_t[:], cosT[:])
            nc.scalar.dma_start(sin_t[:], sinT[:])

            load_w0_done = []

            # v resident per (key-block, head, dh+ones): [128, 16, 8, 65].
            # Memset the whole tile to 1.0 up front: the projection evacs
            # overwrite columns 0-63 of each head, leaving column 64 as the
            # ones column that accumulates the softmax denominator.
            v_sb = cpool.tile([128, MB, HG, DH + 1], BF16, tag="vsb")
            nc.vector.memset(v_sb[:], 1.0)

            # broadcast-matmul stationary, host-built (row0 spreads to
            # output partitions 0-63, row32 to 64-127; other rows are zero)
            onesblk = cpool.tile([64, 128], F32R, tag="onesblk")
            nc.sync.dma_start(onesblk[:], onesd[:])
            # persistent normalize staging: denominator rows 0 and 32 (other
            # rows stay 1.0 so the zero-weight broadcast rows see finite
            # values -- never NaN*0) and the Newton-iteration scratch
            den = at.tile([64, 512], F32, tag="den")
            nc.vector.memset(den[:], 1.0)
            rcp0 = at.tile([64, 512], I32, tag="rcp0")
            rcp_t = at.tile([64, 512], F32, tag="rcpt")
            rcp_u = at.tile([64, 512], F32, tag="rcpu")
            rcp1 = at.tile([64, 512], F32, tag="rcp1")
            rcp2 = at.tile([64, 512], F32R, tag="rcp2")

            # ---- per-pair q/k weight loads ----
            wtiles = {}

            def load_w(p):
                csl = slice(p * 128, (p + 1) * 128)
                ts = {}
                for nm, wd in (("q", wq), ("k", wk)):
                    t = wpool.tile([128, KD, 128], BF16, tag=f"w{nm}")
                    nc.gpsimd.dma_start(
                        t[:], wd.rearrange("(c p) i -> p c i", p=128)[:, :, csl]
                    )
                    ts[nm] = t
                wtiles[p] = ts

            # ---- projection block: qT/kT[:, nb*512:(nb+1)*512] for pair p ----
            def proj_block(p, nm, nb, tgt):
                def emit():
                    nsl = slice(nb * 512, (nb + 1) * 512)
                    pq = ps.tile([128, 512], F32, tag="pq", bufs=2)
                    wt = wtiles[p][nm]
                    for dc in range(KD):
                        nc.tensor.matmul(
                            pq[:], wt[:, dc, :], x_sb[:, dc, nsl],
                            start=(dc == 0), stop=(dc == KD - 1),
                        )
                    qsb = rpool.tile([128, 512], BF16, tag="qsb")
                    nc.vector.tensor_copy(qsb[:], pq[:])
                    # rotate_half: swap 32-row halves within each 64-row head
                    # block (sign folded into the host-negated sin table)
                    tmp = rpool.tile([128, 512], BF16, tag="tmp")
                    for g in range(4):
                        dst = slice(g * 32, (g + 1) * 32)
                        src = slice((g ^ 1) * 32, ((g ^ 1) + 1) * 32)
                        nc.vector.tensor_copy(tmp[dst, :], qsb[src, :])
                    nc.vector.tensor_mul(tmp[:], tmp[:], sin_t[:, nsl])
                    nc.vector.tensor_mul(tgt[:, nsl], qsb[:], cos_t[:, nsl])
                    nc.vector.tensor_add(tgt[:, nsl], tgt[:, nsl], tmp[:])
                return emit

            # ---- v projection block: keys [i*128, (i+1)*128) for all heads ----
            def v_block(i):
                def emit():
                    msl = slice(i * 128, (i + 1) * 128)
                    pv = ps.tile([128, 512], F32, tag="pq", bufs=2)
                    for dc in range(KD):
                        nc.tensor.matmul(
                            pv[:], x_sb[:, dc, msl], wv_t[:, dc, :],
                            start=(dc == 0), stop=(dc == KD - 1),
                        )
                    nc.vector.tensor_copy(v_sb[:, i, :, 0:DH], pv[:])
                return emit

            # ---- output projection block (one 128-query row block, one
            #      512-wide dim half) ----
            otn = [
                at.tile([128, NB, 512], BF16, tag=f"otn{p}", name=f"otn{p}")
                for p in range(PAIRS)
            ]

            def outproj_block(nb, dh):
                def emit():
                    q4, r4 = divmod(nb, 4)
                    nsl = slice(nb * 128, (nb + 1) * 128)
                    po = ps.tile([128, 512], F32, tag="pq", bufs=2)
                    for c in range(PAIRS):
                        nc.tensor.matmul(
                            po[:],
                            otn[c][:, q4, r4 * 128 : (r4 + 1) * 128],
                            wo_t[:, c, dh * 512 : (dh + 1) * 512],
                            start=(c == 0), stop=(c == PAIRS - 1),
                        )
                    ost = iopool.tile([128, 512], F32, tag="ost", bufs=3)
                    nc.vector.tensor_copy(ost[:], po[:])
                    nc.sync.dma_start(out[nsl, dh * 512 : (dh + 1) * 512], ost[:])
                return emit

            # ---- attention pipeline ----
            fillers = []
            avq = []

            def drain_fillers(k):
                for _ in range(k):
                    if fillers:
                        fillers.pop(0)()

            def drain_avq(cap, lag=0):
                pops = max(0, min(cap, len(avq) - lag))
                for _ in range(pops):
                    avq.pop(0)()

            def normalize(p, qi, qd):
                # stage both heads' denominator rows at partitions 0 and 32,
                # take the reciprocal there with a bit-magic seed plus two
                # Newton steps (stock DVE ops only; the per-op cost is
                # per-lane so the 33-row tile costs the same as one row),
                # then spread both rows across 128 partitions with one K=64
                # broadcast matmul.
                ot0, ot1 = qd["ot"]
                nc.vector.tensor_copy(den[0:1, :], ot0[64:65, :])
                nc.vector.tensor_copy(den[32:33, :], ot1[64:65, :])
                sl = slice(0, 33)
                nc.vector.tensor_scalar(
                    rcp0[sl, :], den[sl, :].bitcast(I32), 0x7EF312AC, -1,
                    mybir.AluOpType.subtract, mybir.AluOpType.mult,
                )
                r0 = rcp0[sl, :].bitcast(F32)
                nc.vector.tensor_mul(rcp_t[sl, :], den[sl, :], r0)
                nc.vector.tensor_scalar(
                    rcp_u[sl, :], rcp_t[sl, :], -1.0, 2.0,
                    mybir.AluOpType.mult, mybir.AluOpType.add,
                )
                nc.vector.tensor_mul(rcp1[sl, :], r0, rcp_u[sl, :])
                nc.vector.tensor_mul(rcp_t[sl, :], den[sl, :], rcp1[sl, :])
                nc.vector.tensor_scalar(
                    rcp_u[sl, :], rcp_t[sl, :], -1.0, 2.0,
                    mybir.AluOpType.mult, mybir.AluOpType.add,
                )
                nc.vector.tensor_mul(rcp2[sl, :], rcp1[sl, :], rcp_u[sl, :])
                bc = ps.tile([128, 512], F32, tag="pq", bufs=2)
                nc.tensor.matmul(
                    bc[:], onesblk[0:33, :], rcp2[0:33, :],
                    start=True, stop=True,
                )
                bcsb = at.tile([128, 512], F32, tag="bcsb", bufs=2)
                nc.vector.tensor_copy(bcsb[:], bc[:])
                # fused PSUM-evacuate + normalize (otn rows 0-63 = head 2p,
                # rows 64-127 = head 2p+1)
                nc.vector.tensor_mul(
                    otn[p][0:64, qi, :], ot0[0:64, :], bcsb[0:64, :]
                )
                nc.vector.tensor_mul(
                    otn[p][64:128, qi, :], ot1[0:64, :], bcsb[64:128, :]
                )

            def av_group(p, qi, mb2, qd):
                def emit():
                    if mb2 == 0:
                        qd["ot"] = [
                            ps.tile([128, 512], F32, tag="ot", bufs=2, name=f"ot{j}")
                            for j in range(2)
                        ]
                    pts = qd.pop(("pt", mb2))
                    for j in range(2):
                        pt, is_sch = pts[j]
                        for hm in range(2):
                            mb = 2 * mb2 + hm
                            rhs = pt[:, hm, :]
                            if is_sch:
                                rhs = rhs.bitcast(F32R)
                            nc.tensor.matmul(
                                qd["ot"][j][0:65, :],
                                v_sb[:, mb, 2 * p + j, :],
                                rhs,
                                start=(mb == 0), stop=(mb == MB - 1),
                            )
                    if mb2 == 7:
                        normalize(p, qi, qd)
                        if p == PAIRS - 1:
                            for r4 in range(4):
                                for dh in range(2):
                                    fillers.append(outproj_block(qi * 4 + r4, dh))
                        drain_fillers(2)
                return emit

            # prologue: pair-0 weights first on the gpsimd queue, then the
            # bulkier v/out weights
            load_w(0)
            wv_t = cpool.tile([128, KD, INNER], BF16, tag="wv")
            nc.gpsimd.dma_start(wv_t[:], wv.rearrange("(c p) i -> p c i", p=128))
            wo_t = cpool.tile([128, PAIRS, DIM], BF16, tag="wo")
            nc.gpsimd.dma_start(wo_t[:], wor[:])
            qT = {0: qkpool.tile([128, N], BF16, tag="qT", name="qT0")}
            kT = {0: qkpool.tile([128, N], BF16, tag="kT", name="kT0")}
            for nb in range(NB):
                proj_block(0, "k", nb, kT[0])()
            proj_block(0, "q", 0, qT[0])()
            fillers.extend(v_block(i) for i in range(MB))
            fillers.extend(proj_block(0, "q", nb, qT[0]) for nb in range(1, NB))

            for p in range(PAIRS):
                if p + 1 < PAIRS:
                    load_w(p + 1)
                    qT[p + 1] = qkpool.tile([128, N], BF16, tag="qT", name=f"qT{p+1}")
                    kT[p + 1] = qkpool.tile([128, N], BF16, tag="kT", name=f"kT{p+1}")
                    fillers.extend(
                        proj_block(p + 1, "k", nb, kT[p + 1]) for nb in range(NB)
                    )
                    fillers.extend(
                        proj_block(p + 1, "q", nb, qT[p + 1]) for nb in range(NB)
                    )
                for qi in range(4):
                    n0 = qi * 512
                    qd = {}
                    for mb2 in range(MB // 2):
                        pts = []
                        s_ts = [
                            ps.tile([128, 2, 512], F32, tag="s", bufs=2, name=f"s{j}")
                            for j in range(2)
                        ]
                        # alternate row groups (j0 at partitions 0-63, j1 at
                        # 64-127) so adjacent matmuls overlap in the PE array
                        for hm in range(2):
                            mb = 2 * mb2 + hm
                            msl = slice(mb * 128, (mb + 1) * 128)
                            for j in range(2):
                                psl = slice(64 * j, 64 * (j + 1))
                                nc.tensor.matmul(
                                    s_ts[j][:, hm, :],
                                    kT[p][psl, msl],
                                    qT[p][psl, n0 : n0 + 512],
                                    start=True, stop=True,
                                )
                        for j in range(2):
                            s_t = s_ts[j]
                            is_sch = (p, qi, mb2, j) in SCHRAUDOLPH
                            if is_sch:
                                pt = ptpool.tile(
                                    [128, 2, 512], I32, tag="pti", bufs=6
                                )
                                nc.vector.tensor_scalar(
                                    pt[:], s_t[:], SCH_A, SCH_B,
                                    mybir.AluOpType.mult, mybir.AluOpType.add,
                                )
                            else:
                                pt = ptpool.tile([128, 2, 512], BF16, tag="pt")
                                nc.scalar.activation(pt[:], s_t[:], EXP, scale=SCALE)
                            pts.append((pt, is_sch))
                        qd[("pt", mb2)] = pts
                        avq.append(av_group(p, qi, mb2, qd))
                        # interleave: prologue drains fast, steady state slow
                        if p == 0 and qi == 0:
                            drain_fillers(3)
                        elif p == PAIRS - 1:
                            drain_fillers(1)
                        elif mb2 % 3 == 2:
                            drain_fillers(1)
                        if p == PAIRS - 1:
                            drain_avq(2, lag=1)
                        else:
                            drain_avq(1, lag=3)

            # tail: remaining attention groups, then output projection
            drain_avq(len(avq))
            drain_fillers(len(fillers))
            assert not avq and not fillers

    return nc


_CACHED = {}


def _get_kernel():
    if "nc" not in _CACHED:
        _CACHED["nc"] = build_kernel()
    return _CACHED["nc"]


def kernel(x, rotary_emb_x, Wq, Wkv, Wo, bo):
    import ml_dtypes
    from concourse.bass_utils import run_bass_kernel_spmd

    BF = ml_dtypes.bfloat16
    x = np.asarray(x, np.float32)
    rope = np.asarray(rotary_emb_x, np.float32)
    Wq = np.asarray(Wq, np.float32)
    Wkv = np.asarray(Wkv, np.float32)
    Wo = np.asarray(Wo, np.float32)
    bo = np.asarray(bo, np.float32)

    cosT = np.cos(rope).T  # [64, N]
    sinT = np.sin(rope).T
    cosT2 = np.concatenate([cosT, cosT], axis=0)
    sinT2 = np.concatenate([sinT, sinT], axis=0).copy()
    # fold rotate_half's sign into sin: the low half of each 64-row head
    # block multiplies -q_hi
    sinT2[0:32] = -sinT2[0:32]
    sinT2[64:96] = -sinT2[64:96]
    cosT2 = np.ascontiguousarray(cosT2.astype(BF))
    sinT2 = np.ascontiguousarray(sinT2.astype(BF))

    Wk_full = Wkv[:, : H * DH]
    Wv_full = Wkv[:, H * DH :]

    onesd = np.zeros((64, 128), np.float32)
    onesd[0, 0:64] = 1.0
    onesd[32, 64:128] = 1.0

    xTs = [np.ascontiguousarray(x[b].T.astype(BF)) for b in range(B)]
    in_maps = []
    for core in range(N_CORES):
        b, hg = divmod(core, 2)
        isl = slice(hg * INNER, (hg + 1) * INNER)
        in_maps.append(
            {
                "xT": xTs[b],
                "wq": np.ascontiguousarray(Wq[:, isl].astype(BF)),
                "wk": np.ascontiguousarray(Wk_full[:, isl].astype(BF)),
                "wv": np.ascontiguousarray(Wv_full[:, isl].astype(BF)),
                "wo": np.ascontiguousarray(Wo[isl, :].astype(BF)),
                "cosT": cosT2,
                "sinT": sinT2,
                "onesd": onesd,
            }
        )

    nc = _get_kernel()
    _CACHED["in_maps"] = in_maps
    res = run_bass_kernel_spmd(nc, in_maps, list(range(N_CORES)))
    outs = [res.results[i]["out"] for i in range(N_CORES)]
    full = np.stack(
        [outs[2 * b] + outs[2 * b + 1] + bo for b in range(B)], axis=0
    )
    return full


# revision 14
# speedup vs baseline: 1.2364x; 1.0029x over previous
"""Multi-head self-attention (RoPE + softmax + out-proj) for Trainium2,
sharded over 8 NeuronCores: data-parallel over batch (4) x tensor-parallel
over heads (2 groups of 8). Each core computes q/k/v projections for its
head group, attention, and a partial output projection; the host sums the
two partials per batch and adds the bias.

v2 design (bf16 datapath, resident operands, balanced engines):
  - All matmul operands are bf16 (same PE stream rate as f32r, half the
    SBUF/DMA bytes; DVE elementwise gets the 2x/4x 16-bit perf modes).
  - x^T stays resident in SBUF (loaded once; no DRAM re-reads per pair);
    v is projected once into a per-head SBUF layout [key, head, 65] whose
    65th column is a ones column, so the attention matmul accumulates the
    softmax denominator for free (M=65 matmuls).
  - RoPE's rotate_half is four 32-partition shifted bf16 copies (4x DVE
    mode) off a single PSUM evacuation; the sign is folded into a
    host-negated sin table.
  - Scores are computed transposed (S^T[keys, queries]) with K=64 matmuls
    row-group-packed two heads at a time; exp runs on the scalar engine
    straight out of PSUM in 1024-wide instructions (scale folded in).
  - Softmax normalization: reciprocal_approx_fast on the denominator rows,
    one K=2 broadcast matmul per quarter to spread both heads' reciprocals
    across 128 partitions, and the PSUM evacuation of the attention output
    is fused with the normalize multiply (no staging spill).
  - Software pipeline: per 512-query quarter, 8 slots of
    {scores, exp, av(lagged one quarter), filler}; fillers carry the next
    pair's projections, the v projection (first quarter), and the output
    projection (last pair), keeping the PE dense so HAM stays warm.
"""

import numpy as np

import concourse.bass as bass
import concourse.mybir as mybir
import concourse.tile as tile

B, N, DIM, H, DH = 4, 2048, 1024, 16, 64
SCALE = DH**-0.5
N_CORES = 8
HG = 8  # heads per core
INNER = HG * DH  # 512
PAIRS = 4  # head pairs per core
NB = 4  # 512-wide query/key blocks
MB = 16  # 128-wide key blocks
KD = DIM // 128  # contraction chunks

F32 = mybir.dt.float32
F32R = mybir.dt.float32r
BF16 = mybir.dt.bfloat16
I32 = mybir.dt.int32
EXP = mybir.ActivationFunctionType.Exp

# Schraudolph exp offload to the vector engine: set of (p, qi, mb2, j)
# score tiles whose exp is computed as bitcast(int32(x*A + B)) on the DVE
# instead of the scalar engine (which is otherwise the bottleneck).
SCHRAUDOLPH = set()
SCH_A = SCALE * (1 << 23) / np.log(2.0)
SCH_B = float(127 * (1 << 23)) - 366393.0

MAX_WAITS = 1
WARMUP = True
SPLIT_FILLERS = False


def _split_excess_waits(nc):
    """This walrus build rejects >1 semaphore wait per instruction; hoist
    excess waits onto nops inserted before the instruction on its engine."""
    import bass_rust

    for f in nc.m.functions:
        for bb in f.blocks:
            il = bb.instructions
            i = 0
            while i < len(il):
                inst = il[i]
                si = inst.sync_info
                if si is not None and si.on_wait and len(si.on_wait) > MAX_WAITS:
                    waits = list(si.on_wait)
                    si.on_wait = waits[:MAX_WAITS]
                    rest = waits[MAX_WAITS:]
                    eng = nc.engines[inst.engine]
                    insert_at = i
                    for j in range(0, len(rest), MAX_WAITS):
                        b = eng.nop(nofuse=True, hint="wait_split")
                        ni = b.ins
                        tail = nc.cur_bb.bb.instructions
                        assert tail[-1] is ni
                        tail.pop()
                        nsi = ni.sync_info
                        if nsi is None:
                            ni.sync_info = bass_rust.SyncInfo(
                                on_wait=rest[j : j + MAX_WAITS], on_update=[]
                            )
                        else:
                            nsi.on_wait = rest[j : j + MAX_WAITS]
                        il.insert(insert_at, ni)
                        insert_at += 1
                        i += 1
                i += 1


class _FixedTileContext(tile.TileContext):
    def __exit__(self, exc_type, exc_val, exc_tb):
        res = super().__exit__(exc_type, exc_val, exc_tb)
        if exc_type is None:
            _split_excess_waits(self.nc)
        return res


def build_kernel():
    nc = bass.Bass()
    xT = nc.dram_tensor("xT", [DIM, N], BF16, kind="ExternalInput")
    wq = nc.dram_tensor("wq", [DIM, INNER], BF16, kind="ExternalInput")
    wk = nc.dram_tensor("wk", [DIM, INNER], BF16, kind="ExternalInput")
    wv = nc.dram_tensor("wv", [DIM, INNER], BF16, kind="ExternalInput")
    wo = nc.dram_tensor("wo", [INNER, DIM], BF16, kind="ExternalInput")
    cosT = nc.dram_tensor("cosT", [128, N], BF16, kind="ExternalInput")
    sinT = nc.dram_tensor("sinT", [128, N], BF16, kind="ExternalInput")
    onesd = nc.dram_tensor("onesd", [64, 128], F32R, kind="ExternalInput")
    out = nc.dram_tensor("out", [N, DIM], F32, kind="ExternalOutput")

    xTr = xT.rearrange("(c p) n -> p c n", p=128)
    wor = wo.rearrange("(c p) d -> p c d", p=128)

    with _FixedTileContext(nc) as tc:
        with (
            tc.tile_pool(name="const", bufs=1) as cpool,
            tc.tile_pool(name="w", bufs=2) as wpool,
            tc.tile_pool(name="qk", bufs=2) as qkpool,
            tc.tile_pool(name="rope", bufs=3) as rpool,
            tc.tile_pool(name="pt", bufs=20) as ptpool,
            tc.tile_pool(name="at", bufs=1) as at,
            tc.tile_pool(name="io", bufs=1) as iopool,
            tc.tile_pool(name="ps", space=bass.MemorySpace.PSUM, bufs=1) as ps,
        ):
            # ---- resident constants / activations ----
            # x first, split across the two HWDGE queues (sync + scalar) so
            # the projection chains aren't gated on one serialized queue
            onesblk = cpool.tile([64, 128], F32R, tag="onesblk")
            nc.sync.dma_start(onesblk[:], onesd[:])
            x_sb = cpool.tile([128, KD, N], BF16, tag="x")
            x_engs = [nc.sync, nc.scalar, nc.gpsimd]
            for dc in range(KD):
                eng = x_engs[dc % 3]
                eng.dma_start(x_sb[:, dc, 0:1024], xTr[:, dc, 0:1024])
                eng.dma_start(x_sb[:, dc, 1024:N], xTr[:, dc, 1024:N])
            cos_t = cpool.tile([128, N], BF16, tag="cos")
            sin_t = cpool.tile([128, N], BF16, tag="sin")
            nc.sync.dma_start(cos_t[:], cosT[:])
            nc.scalar.dma_start(sin_t[:], sinT[:])

            load_w0_done = []

            # v resident per (key-block, head, dh+ones): [128, 16, 8, 65].
            # Memset the whole tile to 1.0 up front: the projection evacs
            # overwrite columns 0-63 of each head, leaving column 64 as the
            # ones column that accumulates the softmax denominator.
            v_sb = cpool.tile([128, MB, HG, DH + 1], BF16, tag="vsb")
            nc.vector.memset(v_sb[:], 1.0)

            # persistent normalize staging: denominator rows 0 and 32 (other
            # rows stay 1.0 so the zero-weight broadcast rows see finite
            # values -- never NaN*0) and the Newton-iteration scratch
            den = at.tile([64, 512], F32, tag="den")
            nc.vector.memset(den[:], 1.0)
            rcp0 = at.tile([64, 512], I32, tag="rcp0")
            rcp_t = at.tile([64, 512], F32, tag="rcpt")
            rcp_u = at.tile([64, 512], F32, tag="rcpu")
            rcp1 = at.tile([64, 512], F32, tag="rcp1")
            rcp2 = at.tile([64, 512], F32R, tag="rcp2")

            # ---- per-pair q/k weight loads ----
            wtiles = {}

            def load_w(p):
                csl = slice(p * 128, (p + 1) * 128)
                ts = {}
                for nm, wd in (("q", wq), ("k", wk)):
                    t = wpool.tile([128, KD, 128], BF16, tag=f"w{nm}")
                    nc.gpsimd.dma_start(
                        t[:], wd.rearrange("(c p) i -> p c i", p=128)[:, :, csl]
                    )
                    ts[nm] = t
                wtiles[p] = ts

            # ---- projection block, split into two half-chain emitters so
            #      fillers stay fine-grained (~0.9us of PE work each) ----
            def proj_block_halves(p, nm, nb, tgt):
                st = {}

                def emit_a():
                    nsl = slice(nb * 512, (nb + 1) * 512)
                    pq = ps.tile([128, 512], F32, tag="pq", bufs=2)
                    st["pq"] = pq
                    wt = wtiles[p][nm]
                    for dc in range(4):
                        nc.tensor.matmul(
                            pq[:], wt[:, dc, :], x_sb[:, dc, nsl],
                            start=(dc == 0), stop=False,
                        )

                def emit_b():
                    nsl = slice(nb * 512, (nb + 1) * 512)
                    pq = st.pop("pq")
                    wt = wtiles[p][nm]
                    for dc in range(4, KD):
                        nc.tensor.matmul(
                            pq[:], wt[:, dc, :], x_sb[:, dc, nsl],
                            start=False, stop=(dc == KD - 1),
                        )
                    qsb = rpool.tile([128, 512], BF16, tag="qsb")
                    nc.vector.tensor_copy(qsb[:], pq[:])
                    # rotate_half: swap 32-row halves within each 64-row head
                    # block (sign folded into the host-negated sin table)
                    tmp = rpool.tile([128, 512], BF16, tag="tmp")
                    for g in range(4):
                        dst = slice(g * 32, (g + 1) * 32)
                        src = slice((g ^ 1) * 32, ((g ^ 1) + 1) * 32)
                        nc.vector.tensor_copy(tmp[dst, :], qsb[src, :])
                    nc.vector.tensor_mul(tmp[:], tmp[:], sin_t[:, nsl])
                    nc.vector.tensor_mul(tgt[:, nsl], qsb[:], cos_t[:, nsl])
                    nc.vector.tensor_add(tgt[:, nsl], tgt[:, nsl], tmp[:])

                return emit_a, emit_b

            # ---- v projection block halves: keys [i*128, (i+1)*128) ----
            def v_block_halves(i):
                st = {}

                def emit_a():
                    msl = slice(i * 128, (i + 1) * 128)
                    pv = ps.tile([128, 512], F32, tag="pq", bufs=2)
                    st["pv"] = pv
                    for dc in range(4):
                        nc.tensor.matmul(
                            pv[:], x_sb[:, dc, msl], wv_t[:, dc, :],
                            start=(dc == 0), stop=False,
                        )

                def emit_b():
                    msl = slice(i * 128, (i + 1) * 128)
                    pv = st.pop("pv")
                    for dc in range(4, KD):
                        nc.tensor.matmul(
                            pv[:], x_sb[:, dc, msl], wv_t[:, dc, :],
                            start=False, stop=(dc == KD - 1),
                        )
                    nc.vector.tensor_copy(v_sb[:, i, :, 0:DH], pv[:])

                return emit_a, emit_b

            # ---- output projection block (one 128-query row block, one
            #      512-wide dim half) ----
            otn = [
                at.tile([128, NB, 512], BF16, tag=f"otn{p}", name=f"otn{p}")
                for p in range(PAIRS)
            ]

            def outproj_block(nb, dh):
                def emit():
                    q4, r4 = divmod(nb, 4)
                    nsl = slice(nb * 128, (nb + 1) * 128)
                    po = ps.tile([128, 512], F32, tag="pq", bufs=2)
                    for c in range(PAIRS):
                        nc.tensor.matmul(
                            po[:],
                            otn[c][:, q4, r4 * 128 : (r4 + 1) * 128],
                            wo_t[:, c, dh * 512 : (dh + 1) * 512],
                            start=(c == 0), stop=(c == PAIRS - 1),
                        )
                    ost = iopool.tile([128, 512], F32, tag="ost", bufs=3)
                    nc.vector.tensor_copy(ost[:], po[:])
                    nc.sync.dma_start(out[nsl, dh * 512 : (dh + 1) * 512], ost[:])
                return emit

            # ---- attention pipeline ----
            fillers = []
            avq = []

            def drain_fillers(k):
                for _ in range(k):
                    if fillers:
                        fillers.pop(0)()

            def drain_avq(cap, lag=0):
                pops = max(0, min(cap, len(avq) - lag))
                for _ in range(pops):
                    avq.pop(0)()

            def normalize(p, qi, qd):
                # stage both heads' denominator rows at partitions 0 and 32,
                # take the reciprocal there with a bit-magic seed plus two
                # Newton steps (stock DVE ops only; the per-op cost is
                # per-lane so the 33-row tile costs the same as one row),
                # then spread both rows across 128 partitions with one K=64
                # broadcast matmul.
                ot0, ot1 = qd["ot"]
                nc.vector.tensor_copy(den[0:1, :], ot0[64:65, :])
                nc.vector.tensor_copy(den[32:33, :], ot1[64:65, :])
                sl = slice(0, 33)
                nc.vector.tensor_scalar(
                    rcp0[sl, :], den[sl, :].bitcast(I32), 0x7EF312AC, -1,
                    mybir.AluOpType.subtract, mybir.AluOpType.mult,
                )
                r0 = rcp0[sl, :].bitcast(F32)
                nc.vector.tensor_mul(rcp_t[sl, :], den[sl, :], r0)
                nc.vector.tensor_scalar(
                    rcp_u[sl, :], rcp_t[sl, :], -1.0, 2.0,
                    mybir.AluOpType.mult, mybir.AluOpType.add,
                )
                nc.vector.tensor_mul(rcp1[sl, :], r0, rcp_u[sl, :])
                nc.vector.tensor_mul(rcp_t[sl, :], den[sl, :], rcp1[sl, :])
                nc.vector.tensor_scalar(
                    rcp_u[sl, :], rcp_t[sl, :], -1.0, 2.0,
                    mybir.AluOpType.mult, mybir.AluOpType.add,
                )
                nc.vector.tensor_mul(rcp2[sl, :], rcp1[sl, :], rcp_u[sl, :])
                bc = ps.tile([128, 512], F32, tag="pq", bufs=2)
                nc.tensor.matmul(
                    bc[:], onesblk[0:33, :], rcp2[0:33, :],
                    start=True, stop=True,
                )
                bcsb = at.tile([128, 512], F32, tag="bcsb", bufs=2)
                nc.vector.tensor_copy(bcsb[:], bc[:])
                # fused PSUM-evacuate + normalize (otn rows 0-63 = head 2p,
                # rows 64-127 = head 2p+1)
                nc.vector.tensor_mul(
                    otn[p][0:64, qi, :], ot0[0:64, :], bcsb[0:64, :]
                )
                nc.vector.tensor_mul(
                    otn[p][64:128, qi, :], ot1[0:64, :], bcsb[64:128, :]
                )

            def av_group(p, qi, mb2, qd):
                def emit():
                    if mb2 == 0:
                        qd["ot"] = [
                            ps.tile([128, 512], F32, tag="ot", bufs=2, name=f"ot{j}")
                            for j in range(2)
                        ]
                    pts = qd.pop(("pt", mb2))
                    for j in range(2):
                        pt, is_sch = pts[j]
                        for hm in range(2):
                            mb = 2 * mb2 + hm
                            rhs = pt[:, hm, :]
                            if is_sch:
                                rhs = rhs.bitcast(F32R)
                            nc.tensor.matmul(
                                qd["ot"][j][0:65, :],
                                v_sb[:, mb, 2 * p + j, :],
                                rhs,
                                start=(mb == 0), stop=(mb == MB - 1),
                            )
                    if mb2 == 7:
                        normalize(p, qi, qd)
                        if p == PAIRS - 1:
                            for r4 in range(4):
                                for dh in range(2):
                                    fillers.append(outproj_block(qi * 4 + r4, dh))
                        drain_fillers(1)
                return emit

            def halves(ab):
                if SPLIT_FILLERS:
                    return list(ab)
                a, b = ab

                def f():
                    a()
                    b()

                return [f]

            # prologue: pair-0 weights first on the gpsimd queue, then the
            # bulkier v/out weights
            load_w(0)
            wv_t = cpool.tile([128, KD, INNER], BF16, tag="wv")
            nc.gpsimd.dma_start(wv_t[:], wv.rearrange("(c p) i -> p c i", p=128))
            wo_t = cpool.tile([128, PAIRS, DIM], BF16, tag="wo")
            nc.gpsimd.dma_start(wo_t[:], wor[:])

            # warm-up matmuls on the (tiny, early-arriving) onesblk tile:
            # keep the PE busy while x streams in so HAM reaches full clock
            # before the real projection chains start
            warm = ps.tile([128, 2, 512], F32, tag="s", bufs=2, name="warm")
            for w in range(48 if WARMUP else 0):
                nc.tensor.matmul(
                    warm[:, 0, 0:128], onesblk[:], onesblk[:, 0:128].bitcast(F32R),
                    start=True, stop=True,
                )

            qT = {0: qkpool.tile([128, N], BF16, tag="qT", name="qT0")}
            kT = {0: qkpool.tile([128, N], BF16, tag="kT", name="kT0")}
            for nb in range(NB):
                a, b = proj_block_halves(0, "k", nb, kT[0])
                a(); b()
            a, b = proj_block_halves(0, "q", 0, qT[0])
            a(); b()
            # early fillers: v halves with the remaining qT0 halves woven in
            # so every quarter's inputs land ahead of its scores/av groups
            ev = [halves(v_block_halves(i)) for i in range(MB)]
            eq = [
                halves(proj_block_halves(0, "q", nb, qT[0]))
                for nb in range(1, NB)
            ]
            early = []
            for i in range(MB):
                early.extend(ev[i])
                if i in (1, 3, 5):
                    early.extend(eq[(i - 1) // 2])
            fillers.extend(early)

            for p in range(PAIRS):
                if p + 1 < PAIRS:
                    load_w(p + 1)
                    qT[p + 1] = qkpool.tile([128, N], BF16, tag="qT", name=f"qT{p+1}")
                    kT[p + 1] = qkpool.tile([128, N], BF16, tag="kT", name=f"kT{p+1}")
                    for nb in range(NB):
                        fillers.extend(
                            halves(proj_block_halves(p + 1, "k", nb, kT[p + 1]))
                        )
                    for nb in range(NB):
                        fillers.extend(
                            halves(proj_block_halves(p + 1, "q", nb, qT[p + 1]))
                        )
                for qi in range(4):
                    n0 = qi * 512
                    qd = {}
                    for mb2 in range(MB // 2):
                        pts = []
                        s_ts = [
                            ps.tile([128, 2, 512], F32, tag="s", bufs=2, name=f"s{j}")
                            for j in range(2)
                        ]
                        # alternate row groups (j0 at partitions 0-63, j1 at
                        # 64-127) so adjacent matmuls overlap in the PE array
                        for hm in range(2):
                            mb = 2 * mb2 + hm
                            msl = slice(mb * 128, (mb + 1) * 128)
                            for j in range(2):
                                psl = slice(64 * j, 64 * (j + 1))
                                nc.tensor.matmul(
                                    s_ts[j][:, hm, :],
                                    kT[p][psl, msl],
                                    qT[p][psl, n0 : n0 + 512],
                                    start=True, stop=True,
                                )
                        for j in range(2):
                            s_t = s_ts[j]
                            is_sch = (p, qi, mb2, j) in SCHRAUDOLPH
                            if is_sch:
                                pt = ptpool.tile(
                                    [128, 2, 512], I32, tag="pti", bufs=6
                                )
                                nc.vector.tensor_scalar(
                                    pt[:], s_t[:], SCH_A, SCH_B,
                                    mybir.AluOpType.mult, mybir.AluOpType.add,
                                )
                            else:
                                pt = ptpool.tile([128, 2, 512], BF16, tag="pt")
                                nc.scalar.activation(pt[:], s_t[:], EXP, scale=SCALE)
                            pts.append((pt, is_sch))
                        qd[("pt", mb2)] = pts
                        avq.append(av_group(p, qi, mb2, qd))
                        # interleave: prologue drains fast, steady state one
                        # half-filler per slot
                        if p == 0 and qi in (0, 1):
                            drain_fillers(3)
                        else:
                            drain_fillers(1)
                        if p == PAIRS - 1:
                            drain_avq(2, lag=1)
                        else:
                            drain_avq(1, lag=3)

            # tail: remaining attention groups, then output projection
            drain_avq(len(avq))
            drain_fillers(len(fillers))
            assert not avq and not fillers

    return nc


_CACHED = {}


def _get_kernel():
    if "nc" not in _CACHED:
        _CACHED["nc"] = build_kernel()
    return _CACHED["nc"]


def kernel(x, rotary_emb_x, Wq, Wkv, Wo, bo):
    import ml_dtypes
    from concourse.bass_utils import run_bass_kernel_spmd

    BF = ml_dtypes.bfloat16
    x = np.asarray(x, np.float32)
    rope = np.asarray(rotary_emb_x, np.float32)
    Wq = np.asarray(Wq, np.float32)
    Wkv = np.asarray(Wkv, np.float32)
    Wo = np.asarray(Wo, np.float32)
    bo = np.asarray(bo, np.float32)

    cosT = np.cos(rope).T  # [64, N]
    sinT = np.sin(rope).T
    cosT2 = np.concatenate([cosT, cosT], axis=0)
    sinT2 = np.concatenate([sinT, sinT], axis=0).copy()
    # fold rotate_half's sign into sin: the low half of each 64-row head
    # block multiplies -q_hi
    sinT2[0:32] = -sinT2[0:32]
    sinT2[64:96] = -sinT2[64:96]
    cosT2 = np.ascontiguousarray(cosT2.astype(BF))
    sinT2 = np.ascontiguousarray(sinT2.astype(BF))

    Wk_full = Wkv[:, : H * DH]
    Wv_full = Wkv[:, H * DH :]

    onesd = np.zeros((64, 128), np.float32)
    onesd[0, 0:64] = 1.0
    onesd[32, 64:128] = 1.0

    xTs = [np.ascontiguousarray(x[b].T.astype(BF)) for b in range(B)]
    in_maps = []
    for core in range(N_CORES):
        b, hg = divmod(core, 2)
        isl = slice(hg * INNER, (hg + 1) * INNER)
        in_maps.append(
            {
                "xT": xTs[b],
                "wq": np.ascontiguousarray(Wq[:, isl].astype(BF)),
                "wk": np.ascontiguousarray(Wk_full[:, isl].astype(BF)),
                "wv": np.ascontiguousarray(Wv_full[:, isl].astype(BF)),
                "wo": np.ascontiguousarray(Wo[isl, :].astype(BF)),
                "cosT": cosT2,
                "sinT": sinT2,
                "onesd": onesd,
            }
        )

    nc = _get_kernel()
    _CACHED["in_maps"] = in_maps
    res = run_bass_kernel_spmd(nc, in_maps, list(range(N_CORES)))
    outs = [res.results[i]["out"] for i in range(N_CORES)]
    full = np.stack(
        [outs[2 * b] + outs[2 * b + 1] + bo for b in range(B)], axis=0
    )
    return full
